# revision 1
# baseline (speedup 1.0000x reference)
"""Trainium2 Bass kernel for nn_CamAttnCon (topk-masked CAM attention consolidation).

Computation (per sample b):
  w[t]   = cosine(target_embed[b,t,:], fore_rep_encoded[b,:])     (masked where tgt<=0)
  top-k  = indices of the m largest w (m = min(ceil(0.1*seqlen), 51))
  total  = mean over top-m of relu(w[t]) * mean_h(align_attns[2][b,:,t,:])
  out    = minmax-normalize(total)                                 [B, S]

Strategy: pure data-parallel over batch; 4 samples per core on 8 cores.
On-device per sample: cosine via fused DVE/ACT reduce ops over the embedding,
exact top-k selection by rank (pairwise compare + ones-matmul), one-hot
compaction of selected indices, indirect-DMA gather of only the selected
attention rows, and a PSUM-accumulated weighted matmul for the head/topk
reduction. All stages are pipelined per sample.
"""

import os
import sys

sys.path.insert(0, "/opt/trn_rl_repo")

import numpy as np
from contextlib import ExitStack

import concourse.bass as bass
import concourse.bacc as bacc
import concourse.mybir as mybir
import concourse.tile as tile
from concourse.masks import make_identity
from concourse import bass_utils

f32 = mybir.dt.float32
bf16 = mybir.dt.bfloat16
fp16 = mybir.dt.float16
f32r = mybir.dt.float32r
i32 = mybir.dt.int32
AX = mybir.AxisListType
OP = mybir.AluOpType
AF = mybir.ActivationFunctionType

B, T, D, H, S = 32, 512, 512, 8, 196
NCORES = 8
BL = B // NCORES            # 4 samples per core
TC = T // 128               # 4 t-chunks of 128
HS = H * S                  # 1568
KK = int(0.1 * T)           # 51
J = 64                      # padded top-k slot count (>= KK)
EPS_COS = 1e-8
EPS_NORM = 1e-12
NEG_BIG = -1e30

LAST_EXEC_NS = None
LAST_RESULTS = None


def cb(c, b):
    """flat column index for (chunk, sample) pairs in [128, TC*BL] tiles"""
    return c * BL + b


def build_body(ctx, tc, emb, att, fore_bc, tgt, out):
    nc = tc.nc

    # ---------------- pools ----------------
    const = ctx.enter_context(tc.tile_pool(name="const", bufs=1))
    small = ctx.enter_context(tc.tile_pool(name="small", bufs=1))
    embp = ctx.enter_context(tc.tile_pool(name="embp", bufs=8))
    scr = ctx.enter_context(tc.tile_pool(name="scr", bufs=6))
    embr = ctx.enter_context(tc.tile_pool(name="embr", bufs=8))
    wbcp = ctx.enter_context(tc.tile_pool(name="wbcp", bufs=3))
    cmpp = ctx.enter_context(tc.tile_pool(name="cmpp", bufs=4))
    stp = ctx.enter_context(tc.tile_pool(name="stp", bufs=3))
    gatp = ctx.enter_context(tc.tile_pool(name="gatp", bufs=4))

    ps_bc = ctx.enter_context(tc.tile_pool(name="ps_bc", bufs=2, space="PSUM"))
    ps_sm = ctx.enter_context(tc.tile_pool(name="ps_sm", bufs=2, space="PSUM"))
    ps_num = ctx.enter_context(tc.tile_pool(name="ps_num", bufs=1, space="PSUM"))
    ps_xn = ctx.enter_context(tc.tile_pool(name="ps_xn", bufs=1, space="PSUM"))
    ps_pair = ctx.enter_context(tc.tile_pool(name="ps_pair", bufs=1, space="PSUM"))
    ps_tot = ctx.enter_context(tc.tile_pool(name="ps_tot", bufs=1, space="PSUM"))

    # ---------------- constants ----------------
    id128 = const.tile([128, 128], f32, tag="id128")
    make_identity(nc, id128[:])
    onesM = const.tile([1, 128], f32, tag="onesM")
    nc.vector.memset(onesM[:], 1.0)
    # warm the ACT function tables early (overlaps input DMA)
    warm = const.tile([1, 1], f32, tag="warm")
    nc.vector.memset(warm[:], 1.0)
    warm2 = const.tile([1, 1], f32, tag="warm2")
    nc.scalar.sqrt(warm2[:], warm[:])
    nc.scalar.activation(out=warm2[:], in_=warm[:], func=AF.Square)
    nc.scalar.copy(warm2[:], warm[:])
    # bcsel4: lhsT [BL,128] slice c = row c all-ones (partition bcast selector)
    bcsel4 = const.tile([BL, TC * 128], f32, tag="bcsel4")
    nc.gpsimd.memset(bcsel4[:], 0.0)
    nc.gpsimd.affine_select(
        out=bcsel4[:].rearrange("p (blk j) -> p blk j", blk=TC),
        in_=bcsel4[:].rearrange("p (blk j) -> p blk j", blk=TC),
        compare_op=OP.not_equal,
        fill=1.0,
        base=0,
        pattern=[[-1, TC], [0, 128]],
        channel_multiplier=1,
    )

    tv_i = const.tile([128, TC], i32, tag="tv_i")
    nc.gpsimd.iota(tv_i[:], pattern=[[128, TC]], base=0, channel_multiplier=1)
    tv_f = const.tile([128, TC], f32, tag="tv_f")
    nc.vector.tensor_copy(tv_f[:], tv_i[:])

    jv_i = const.tile([128, J], i32, tag="jv_i")
    nc.gpsimd.iota(jv_i[:], pattern=[[1, J]], base=0, channel_multiplier=0)
    jv_f = const.tile([128, J], f32, tag="jv_f")
    nc.vector.tensor_copy(jv_f[:], jv_i[:])

    ten_i = const.tile([BL, KK], i32, tag="ten_i")
    nc.gpsimd.iota(ten_i[:], pattern=[[10, KK]], base=0, channel_multiplier=0)
    ten_f = const.tile([BL, KK], f32, tag="ten_f")
    nc.vector.tensor_copy(ten_f[:], ten_i[:])

    negbig = const.tile([128, TC], f32, tag="negbig")
    nc.vector.memset(negbig[:], NEG_BIG)

    # boff2[:, b] = (T*b, 0): add sample-b row offset to the t row only
    boff2_i = const.tile([2, BL], i32, tag="boff2_i")
    nc.gpsimd.iota(boff2_i[:], pattern=[[T, BL]], base=0, channel_multiplier=0)
    boff2 = const.tile([2, BL], f32, tag="boff2")
    nc.vector.tensor_copy(boff2[:], boff2_i[:])
    pm2_i = const.tile([2, 1], i32, tag="pm2_i")
    nc.gpsimd.iota(pm2_i[:], pattern=[[1, 1]], base=0, channel_multiplier=1)
    pm2 = const.tile([2, 1], f32, tag="pm2")
    nc.vector.tensor_copy(pm2[:], pm2_i[:])
    nc.vector.tensor_scalar(
        out=pm2[:], in0=pm2[:], scalar1=1.0, scalar2=None, op0=OP.is_lt
    )
    nc.vector.tensor_scalar(
        out=boff2[:], in0=boff2[:], scalar1=pm2[:], scalar2=None, op0=OP.mult
    )

    # v2_b variants: per c slot of 2 cols: col 0 = t-values, col 1 = g (late)
    v2t = []
    for b in range(BL):
        v2b = const.tile([128, TC * 2], f32, tag=f"v2_{b}")
        for c in range(TC):
            nc.vector.tensor_copy(v2b[:, c * 2 : c * 2 + 1], tv_f[:, c : c + 1])
        v2t.append(v2b)

    # ---------------- input loads (interleave fore_bc slices with emb) ------
    tgt_rows_i = small.tile([BL, T], i32, tag="tgt_rows_i")
    nc.sync.dma_start(tgt_rows_i[:], tgt[:])

    fore_sel_f = small.tile([128, TC * BL * BL], f32, tag="fore_sel_f")
    nc.scalar.dma_start(fore_sel_f[:], fore_bc[:])
    fore_sel = small.tile([128, TC * BL * BL], f32r, tag="fore_sel")
    nc.vector.tensor_copy(fore_sel[:], fore_sel_f[:])
    embR = emb.rearrange("b (dc p) t -> b p dc t", p=128)
    embt = []
    for b in range(BL):
        etiles = []
        for dc in range(TC):
            e = embp.tile([128, T], f32r, tag="emb")
            nc.sync.dma_start(e[:], embR[b][:, dc, :])
            etiles.append(e)
        embt.append(etiles)

    # ---------------- mask / seqlen / m (independent of embed) ----------------
    tgt_rows_f = small.tile([BL, T], f32, tag="tgt_rows_f")
    nc.vector.tensor_copy(tgt_rows_f[:], tgt_rows_i[:])
    mask_rows = small.tile([BL, T], f32, tag="mask_rows")
    nc.vector.tensor_scalar(
        out=mask_rows[:], in0=tgt_rows_f[:], scalar1=0.0, scalar2=None, op0=OP.is_gt
    )
    nc.vector.memset(mask_rows[:, 0:1], 1.0)
    seqcol = small.tile([BL, 1], f32, tag="seqcol")
    nc.vector.tensor_reduce(seqcol[:], mask_rows[:], axis=AX.X, op=OP.add)


    # m = min(ceil(0.1*seqlen), KK) = sum_i [10*i < seqlen], i in [0, KK)
    mcnt = small.tile([BL, KK], f32, tag="mcnt")
    nc.vector.tensor_scalar(
        out=mcnt[:], in0=ten_f[:], scalar1=seqcol[:], scalar2=None, op0=OP.is_lt
    )
    mcol = small.tile([BL, 1], f32, tag="mcol")
    nc.vector.tensor_reduce(mcol[:], mcnt[:], axis=AX.X, op=OP.add)

    # NOTE: the reference's per-sample scales (1/m, 1/H, 1/yn) are all positive
    # per-sample constants; min-max normalization cancels them exactly, so we
    # skip them entirely and only need m for the top-m cutoff.
    # mbc: m broadcast to all 128 partitions (for the rank < m compare)
    mr_ps = ps_sm.tile([1, BL], f32, tag="tsm")
    nc.tensor.transpose(mr_ps[:], mcol[:], id128[0:BL, 0:BL])
    mrow = small.tile([1, BL], f32, tag="mrow")
    nc.vector.tensor_copy(mrow[:], mr_ps[:])
    mbc_ps = ps_sm.tile([128, BL], f32, tag="tsm")
    nc.tensor.matmul(out=mbc_ps[:], lhsT=onesM[:], rhs=mrow[:], start=True, stop=True)
    mbc = small.tile([128, BL], f32, tag="mbc")
    nc.vector.tensor_copy(mbc[:], mbc_ps[:])

    # ---------------- per-sample pipeline ----------------
    tot_ps = ps_tot.tile([BL, S], f32, tag="tot")

    # -------- cosine via PE: f32r col-selector matmuls (rounding producers) --
    onesel_f = const.tile([128, BL * BL], f32, tag="onesel_f")
    nc.vector.memset(onesel_f[:], 0.0)
    for b in range(BL):
        nc.vector.memset(onesel_f[:, b * BL + b : b * BL + b + 1], 1.0)
    onesel = const.tile([128, BL * BL], f32r, tag="onesel")
    nc.vector.tensor_copy(onesel[:], onesel_f[:])
    num_ps = ps_num.tile([BL, T], f32, tag="num")
    xn2_ps = ps_xn.tile([BL, T], f32, tag="xn2")
    for b in range(BL):
        for dc in range(TC):
            x = embt[b][dc][:]
            xr = embt[b][dc]
            sq = scr.tile([128, T], f32r, tag="scr")
            nc.scalar.activation(out=sq[:], in_=x, func=AF.Square)
            fsel = fore_sel[:, (dc * BL + b) * BL : (dc * BL + b) * BL + BL]
            nc.tensor.matmul(
                out=num_ps[:],
                lhsT=fsel,
                rhs=xr[:],
                start=(b == 0 and dc == 0),
                stop=(b == BL - 1 and dc == TC - 1),
            )
            nc.tensor.matmul(
                out=xn2_ps[:],
                lhsT=onesel[:, b * BL : (b + 1) * BL],
                rhs=sq[:],
                start=(b == 0 and dc == 0),
                stop=(b == BL - 1 and dc == TC - 1),
            )

    # ---------------- w rows [BL, T] then transpose to T-layout --------------
    xn_rows = small.tile([BL, T], f32, tag="xn_rows")
    nc.scalar.sqrt(xn_rows[:], xn2_ps[:])
    rxn_rows = small.tile([BL, T], f32, tag="rxn_rows")
    nc.vector.reciprocal(rxn_rows[:], xn_rows[:])
    wraw_rows = small.tile([BL, T], f32, tag="wraw_rows")
    nc.vector.tensor_tensor(wraw_rows[:], num_ps[:], rxn_rows[:], op=OP.mult)
    mask_i = small.tile([BL, T], i32, tag="mask_i")
    nc.vector.tensor_copy(mask_i[:], mask_rows[:])
    negrow = small.tile([BL, T], f32, tag="negrow")
    nc.vector.memset(negrow[:], NEG_BIG)
    w_rows = small.tile([BL, T], f32, tag="w_rows")
    nc.vector.select(w_rows[:], mask_i[:], wraw_rows[:], negrow[:])
    wT = small.tile([128, TC * BL], f32, tag="wT")
    for c in range(TC):
        pswt = ps_sm.tile([128, BL], f32, tag="tsm")
        nc.tensor.transpose(pswt[:], w_rows[:, c * 128 : (c + 1) * 128], id128[0:BL, 0:BL])
        nc.vector.tensor_copy(wT[:, c * BL : (c + 1) * BL], pswt[:])

    for b in range(BL):
        wT_b = wT[:].rearrange("p (c b) -> p c b", b=BL)[:, :, b]

        # --- broadcast w[b,:] to all partitions via row-selector matmul ---
        wbc_ps = ps_bc.tile([128, T], f32, tag="bc")
        nc.tensor.matmul(
            out=wbc_ps[:],
            lhsT=bcsel4[:, b * 128 : (b + 1) * 128],
            rhs=w_rows[:],
            start=True,
            stop=True,
        )
        wbc_sb = wbcp.tile([128, T], f32, tag="wbc")
        nc.scalar.copy(wbc_sb[:], wbc_ps[:])

        # --- rank directly in T-layout:  rankT[q,c] = #{t' : w[t'] > w[c*128+q]}
        # one fused compare+accumulate per chunk (accum_out sums the 0/1 row)
        rankT_b = small.tile([128, TC], f32, tag=f"rankT{b}")
        for c in range(TC):
            cmp_bf = cmpp.tile([128, T], bf16, tag="cmp")
            nc.vector.tensor_scalar(
                out=cmp_bf[:],
                in0=wbc_sb[:],
                scalar1=wT_b[:, c : c + 1],
                scalar2=None,
                op0=OP.is_gt,
                op1=OP.add,
                accum_out=rankT_b[:, c : c + 1],
            )
        selT_b = small.tile([128, TC], f32, tag=f"selT{b}")
        nc.vector.tensor_scalar(
            out=selT_b[:],
            in0=rankT_b[:],
            scalar1=mbc[:, b : b + 1],
            scalar2=None,
            op0=OP.is_lt,
        )
        gT_b = small.tile([128, TC], f32, tag=f"gT{b}")
        nc.vector.scalar_tensor_tensor(
            out=gT_b[:],
            in0=wT_b[:],
            scalar=0.0,
            in1=selT_b[:],
            op0=OP.max,
            op1=OP.mult,
        )
        v2b = v2t[b]
        nc.vector.tensor_copy(
            v2b[:].rearrange("p (c two) -> p c two", two=2)[:, :, 1], gT_b[:]
        )

        # --- one-hot compaction: stak2 rows = (compact t, compact g) ---
        # one fused is_equal over all chunks via broadcast APs
        st4 = stp.tile([128, TC * J], f32, tag="st")
        nc.vector.tensor_tensor(
            out=st4[:].rearrange("p (c j) -> p c j", c=TC),
            in0=jv_f[:].unsqueeze(1).broadcast_to([128, TC, J]),
            in1=rankT_b[:].unsqueeze(2).broadcast_to([128, TC, J]),
            op=OP.is_equal,
        )
        stak2 = ps_sm.tile([2, J], f32, tag="tsm")
        for c in range(TC):
            nc.tensor.matmul(
                out=stak2[:],
                lhsT=v2b[:, c * 2 : (c + 1) * 2],
                rhs=st4[:, c * J : (c + 1) * J],
                start=(c == 0),
                stop=(c == TC - 1),
            )
        stack2 = stp.tile([2, J], f32, tag="stack2")
        nc.vector.tensor_scalar(
            out=stack2[:],
            in0=stak2[:],
            scalar1=boff2[:, b : b + 1],
            scalar2=None,
            op0=OP.add,
        )

        # --- transpose to columns; gather this sample's rows ---
        pstP = ps_pair.tile([J, 2], f32, tag="pairT")
        nc.tensor.transpose(pstP[:], stack2[:], id128[0:2, 0:2])
        idx_b = small.tile([J, 1], i32, tag=f"idxP{b}")
        nc.vector.tensor_copy(idx_b[:], pstP[:, 0:1])
        gsel_b = small.tile([J, BL], fp16, tag=f"gselP{b}")
        nc.vector.memset(gsel_b[:], 0.0)
        nc.vector.tensor_copy(gsel_b[:, b : b + 1], pstP[:, 1:2])
        gat_b = gatp.tile([J, HS], fp16, tag="gat")
        nc.gpsimd.indirect_dma_start(
            out=gat_b[:],
            out_offset=None,
            in_=att[:],
            in_offset=bass.IndirectOffsetOnAxis(ap=idx_b[:, 0:1], axis=0),
        )
        for h in range(H):
            nc.tensor.matmul(
                out=tot_ps[:],
                lhsT=gsel_b[:],
                rhs=gat_b[:, h * S : (h + 1) * S],
                start=(b == 0 and h == 0),
                stop=(b == BL - 1 and h == H - 1),
            )

    # ---------------- normalize ----------------
    mn = small.tile([BL, 1], f32, tag="mn")
    nc.vector.tensor_reduce(mn[:], tot_ps[:], axis=AX.X, op=OP.min)
    mx = small.tile([BL, 1], f32, tag="mx")
    nc.vector.tensor_reduce(mx[:], tot_ps[:], axis=AX.X, op=OP.max)
    nc.vector.tensor_tensor(mx[:], mx[:], mn[:], op=OP.subtract)
    nc.vector.tensor_scalar_max(mx[:], mx[:], EPS_NORM)
    rmx = small.tile([BL, 1], f32, tag="rmx")
    nc.vector.reciprocal(rmx[:], mx[:])
    out_sb = small.tile([BL, S], f32, tag="out_sb")
    nc.vector.tensor_scalar(
        out=out_sb[:],
        in0=tot_ps[:],
        scalar1=mn[:],
        scalar2=rmx[:],
        op0=OP.subtract,
        op1=OP.mult,
    )
    nc.sync.dma_start(out[:], out_sb[:])


def build_nc(path=None):
    nc = bacc.Bacc("TRN2", target_bir_lowering=False, debug=False)
    emb = nc.dram_tensor("emb", [BL, D, T], f32r, kind="ExternalInput")
    att = nc.dram_tensor("att", [BL * T, HS], f32, kind="ExternalInput")
    fore_bc = nc.dram_tensor("fore_sel", [128, TC * BL * BL], f32, kind="ExternalInput")
    tgt = nc.dram_tensor("tgt", [BL, T], i32, kind="ExternalInput")
    out = nc.dram_tensor("out", [BL, S], f32, kind="ExternalOutput")
    with ExitStack() as ctx:
        tc = ctx.enter_context(tile.TileContext(nc))
        build_body(
            ctx, tc, emb.ap(), att.ap(), fore_bc.ap(), tgt.ap(), out.ap()
        )
    nc.compile()
    return nc


_NC_CACHE = {}


def get_nc(path=None):
    if "nc" not in _NC_CACHE:
        _NC_CACHE["nc"] = build_nc()
    return _NC_CACHE["nc"]


def _make_fore_sel(fore_sl):
    fs = np.zeros((128, TC * BL * BL), np.float32)
    for dc in range(TC):
        for b in range(BL):
            fs[:, (dc * BL + b) * BL + b] = fore_sl[b, dc * 128 : (dc + 1) * 128]
    return fs


def make_in_maps(fore_rep_encoded, target_embed, align_attns, targets):
    LAYER_ID = 2
    att_l = np.transpose(np.asarray(align_attns[LAYER_ID]), (0, 2, 1, 3))  # [B,T,H,S]
    in_maps = []
    for cidx in range(NCORES):
        sl = slice(cidx * BL, (cidx + 1) * BL)
        fore_sl = np.ascontiguousarray(
            np.asarray(fore_rep_encoded)[sl], dtype=np.float32
        )
        in_maps.append(
            {
                "emb": np.ascontiguousarray(
                    np.swapaxes(np.asarray(target_embed)[sl], 1, 2), dtype=np.float32
                ),
                "att": np.ascontiguousarray(att_l[sl], dtype=np.float32).reshape(
                    BL * T, HS
                ),
                "fore_sel": _make_fore_sel(fore_sl),
                "tgt": np.ascontiguousarray(np.asarray(targets)[sl, :T]).astype(
                    np.int32
                ),
            }
        )
    return in_maps


def kernel(fore_rep_encoded, target_embed, align_attns, targets):
    global LAST_EXEC_NS, LAST_RESULTS
    nc = get_nc()
    in_maps = make_in_maps(fore_rep_encoded, target_embed, align_attns, targets)
    trace = bool(os.environ.get("KERNEL_TRACE"))
    try:
        res = bass_utils.run_bass_kernel_spmd(
            nc, in_maps, core_ids=list(range(NCORES)), trace=trace
        )
    except ModuleNotFoundError:
        # NTFF trace hook unavailable in this environment; run without trace
        os.environ["BASS_NEVER_TRACE"] = "1"
        res = bass_utils.run_bass_kernel_spmd(
            nc, in_maps, core_ids=list(range(NCORES)), trace=False
        )
    LAST_EXEC_NS = res.exec_time_ns
    LAST_RESULTS = res
    return np.concatenate([r["out"] for r in res.results], axis=0)

